# revision 109
# baseline (speedup 1.0000x reference)
"""Causal self-attention (B=2, T=2048, C=1024, H=16, rope) on 8 trn2 cores.

Sharding: core i = (batch b = i // 4, head-group g = i % 4 owning heads 4g..4g+3).
Each core computes its 4 heads' attention and a partial projection (transposed,
bf16); the host sums the 4 head-group partials per batch and adds the biases
(v-bias is folded into a host-side constant since sum(att) == 1).

QKV/V/PV/proj matmuls are bf16 (1 cycle/row at any N). The S matmuls run in
fp8e4 DoubleRow (0.5 cycles/row): rope outputs are stored fp8 in a staging
tile (W columns permuted to [hA-evens | hB-evens | hA-odds | hB-odds] so the
rope sin-swap is 2 tensor ops instead of 4) and two SBUF->SBUF DMAs per q/k
pair repack them into [64, pair-half i, {q,k}, T] tiles whose (partition, i)
pairs are the DoubleRow contraction. x arrives transposed from the host; qk
bias is a per-partition tensor_scalar_add fused into the psum evict; exp pairs
on Act (kept free of DMA traffic); causal mask on DVE; denominators via a ones
column in v (PV matmul row 64); 1/l broadcast via gpsimd partition_broadcast.

Emission WEAVES work at instruction granularity: the attention j-loop of chunk
t is exp(Act)-paced, leaving PE micro-gaps; units of phase A(t+1) (qk m-tiles,
v tiles) and proj (mo tiles) are emitted between j iterations so PE's program
order fills those gaps. ALL proj work is deferred into chunk 3 (its j-loop is
the longest exp-bound stretch and otherwise PE-starved, while earlier chunks
host phase A and are PE-bound); yp bufs=4 keeps all four y chunks live.
Phase A is front-loaded (finishes ~80% in) so each boundary's rope + repack
chain lands before the next chunk's first S. DMA queue discipline: each
dma_start holds its queue sequencer in order for a fixed ~1-2us plus
transfer, so the SP/HWDGE queue carries the latency-critical stream (x,
repack, paired out DMAs) in need/readiness order, x prefetch is emitted
mid-chunk so its ~5us queue hold cannot delay boundary repacks, and bulk
constants ride the Pool/SWDGE queue. Boundary-critical psum evicts (chunk-0
m2/m3) route via DVE, away from the exp-backlogged Act queue; the held-back
drain units (mo 7 of each chunk-3 proj generator) evict via Act, which is
idle post-exp while DVE carries the tail's normalize chain.
"""

import numpy as np

B, T, C, H = 2, 2048, 1024, 16
HS = C // H            # 64
HPC = H // 4           # 4 heads per core
NCORES = 8
TCH = 512              # t/q chunk size
NCH = T // TCH         # 4 chunks
NSLAB = T // 128       # 16 t-slabs

_cache = {}
last_results = None    # BassKernelResults of the most recent run (for test.py)


def _build():
    import concourse.bacc as bacc
    import concourse.mybir as mybir
    import concourse.tile as tile

    F32 = mybir.dt.float32
    F32R = mybir.dt.float32r
    BF16 = mybir.dt.bfloat16
    F8 = mybir.dt.float8e4
    AF = mybir.ActivationFunctionType
    DR = mybir.MatmulPerfMode.DoubleRow

    nc = bacc.Bacc("TRN2", target_bir_lowering=False, debug=False,
                   num_devices=NCORES)

    xt_in = nc.dram_tensor("xt_in", (C, T), BF16, kind="ExternalInput")
    # wqk arrives pre-packed in the SBUF layout [p, m, s, c] so its DMAs are
    # fully contiguous per partition (column-sliced reads cost 2x on the bus)
    wqk = nc.dram_tensor("wqk", (128, 4096), BF16, kind="ExternalInput")
    bqk_c = nc.dram_tensor("bqk_c", (128, 4), F32, kind="ExternalInput")
    wv = nc.dram_tensor("wv", (128, 2048), BF16, kind="ExternalInput")
    wp = nc.dram_tensor("wp", (256, C), BF16, kind="ExternalInput")
    cos_in = nc.dram_tensor("cos_in", (128, T), BF16, kind="ExternalInput")
    sin_in = nc.dram_tensor("sin_in", (128, T), BF16, kind="ExternalInput")
    cmask = nc.dram_tensor("cmask", (128, 128), BF16, kind="ExternalInput")
    out_t = nc.dram_tensor("out_t", (C, T), BF16, kind="ExternalOutput")

    with tile.TileContext(nc) as tc:
        with (
            tc.tile_pool(name="const", bufs=1) as const,
            tc.tile_pool(name="xp", bufs=4) as xp,
            tc.tile_pool(name="work", bufs=8) as work,
            tc.tile_pool(name="ep", bufs=16) as ep,
            tc.tile_pool(name="yp", bufs=4) as yp,
            tc.tile_pool(name="ost", bufs=6) as ost,
            tc.tile_pool(name="ps_a", bufs=2, space="PSUM") as ps_a,
            tc.tile_pool(name="ps_s", bufs=2, space="PSUM") as ps_s,
            tc.tile_pool(name="ps_o", bufs=2, space="PSUM") as ps_o,
        ):
            # ---- chunk-0 x as per-slab DMAs so the first qk matmuls can
            # start as soon as slab 0 + the first wqk m-tile land ----
            # m-tile-major wqk; the m0 tile rides the SP queue (polled first
            # by the descriptor generator) so it is the first DMA on the bus
            # The whole startup-critical chain rides the SP queue in priority
            # order (the DMA bus is serial; other queues' items interleave,
            # so they only get small or late transfers)
            xts = [xp.tile([128, 8, TCH], BF16, tag="xt", name=f"xt{c}")
                   for c in range(NCH)]
            wqk_sb = const.tile([128, 4, 8, 128], BF16)
            # first m-tile + first x slabs split fine so the first qk matmuls
            # start after ~2 small transfers instead of 2 big ones
            nc.sync.dma_start(
                wqk_sb[:, 0, 0:2, :],
                wqk.ap()[:, 0:256].rearrange("p (s c) -> p s c", s=2))
            nc.sync.dma_start(
                xts[0][:, 0:2, :],
                xt_in.ap()[0:256, 0:TCH]
                .rearrange("(s p) m -> p s m", p=128))
            nc.sync.dma_start(
                wqk_sb[:, 0, 2:8, :],
                wqk.ap()[:, 256:1024].rearrange("p (s c) -> p s c", s=6))
            for q4 in range(1, 4):
                nc.sync.dma_start(
                    xts[0][:, 2 * q4:2 * q4 + 2, :],
                    xt_in.ap()[256 * q4:256 * q4 + 256, 0:TCH]
                    .rearrange("(s p) m -> p s m", p=128))
            for m in range(1, 4):
                nc.sync.dma_start(
                    wqk_sb[:, m, :, :],
                    wqk.ap()[:, m * 1024:(m + 1) * 1024]
                    .rearrange("p (s c) -> p s c", s=8))
            wv_sb = const.tile([128, 8, 256], BF16)
            nc.sync.dma_start(wv_sb[:],
                              wv.ap().rearrange("p (s c) -> p s c", s=8))
            # gpsimd queue: tiny transfers first, then chunk-0/1 cos/sin
            bqk_sb = const.tile([128, 4], F32)
            nc.gpsimd.dma_start(bqk_sb[:], bqk_c[:, :])
            msk_sb = const.tile([128, 128], BF16)
            nc.gpsimd.dma_start(msk_sb[:], cmask[:, :])
            cos_sb = const.tile([128, T], BF16)
            sin_sb = const.tile([128, T], BF16)
            nc.gpsimd.dma_start(cos_sb[:, 0:TCH], cos_in[:, 0:TCH])
            nc.gpsimd.dma_start(sin_sb[:, 0:TCH], sin_in[:, 0:TCH])
            wp_sb = const.tile([128, 2, C], BF16)

            # ---- persistent activations ----
            # fp8 q/k packed for DoubleRow S: [partition 32*hh+p, pair-half i,
            # m-tile {q01,k01,q23,k23}, T]; contraction pair (p, i) = hs
            # coords (even_p, odd_p) of head hh
            qkpk = const.tile([64, 2, 4, T], F8, name="qkpk", tag="qkpk")
            # v with ones column: [t-slab-part, slab, head, 65]
            v_sb = const.tile([128, NSLAB, HPC, 65], BF16)
            ones128 = const.tile([128, 64], F32)
            nc.gpsimd.memset(ones128[:], 1.0)
            nc.vector.tensor_copy(
                v_sb[:, :, :, 64],
                ones128[:, 0:64].rearrange("p (s h) -> p s h", s=NSLAB))

            swap = [(0, 64, 64, 128), (64, 128, 0, 64)]
            stg_hold = {}

            def qk_unit(tcH, m, act_evict):
                tcols = slice(tcH * TCH, (tcH + 1) * TCH)
                pqk = ps_a.tile([128, TCH], F32, tag="a")
                for s in range(8):
                    nc.tensor.matmul(pqk[:], wqk_sb[:, m, s, :],
                                     xts[tcH][:, s, :], start=(s == 0),
                                     stop=(s == 7))
                tQr = work.tile([128, TCH], BF16, tag="tQr")
                if act_evict:
                    nc.scalar.add(tQr[:], pqk[:], bqk_sb[:, m:m + 1])
                else:
                    nc.vector.tensor_scalar_add(tQr[:], pqk[:],
                                                bqk_sb[:, m:m + 1])
                tQc = work.tile([128, TCH], BF16, tag="tQc")
                nc.vector.tensor_mul(tQc[:], tQr[:], cos_sb[:, tcols])
                tQs = work.tile([128, TCH], BF16, tag="tQs")
                for (a0, a1, b0, b1) in swap:
                    nc.vector.tensor_mul(tQs[a0:a1, :], tQr[b0:b1, :],
                                         sin_sb[b0:b1, tcols])
                if m == 0:
                    stg_hold[tcH] = work.tile([128, 4, TCH], F8, tag="stg4",
                                              name="stg4")
                stg4 = stg_hold[tcH]
                nc.vector.tensor_add(stg4[:, m, :], tQc[:], tQs[:])
                if m % 2 == 1:
                    # repack roped q/k per head pair: partitions
                    # [evens(64) | odds(64)] fold into the free pair-half dim
                    for i in range(2):
                        nc.sync.dma_start(
                            qkpk[:, i, m - 1:m + 1, tcols],
                            stg4[64 * i:64 * i + 64, m - 1:m + 1, :])
                    if m == 3:
                        del stg_hold[tcH]

            def v_unit(tcH, ts, act_evict):
                pv = ps_a.tile([128, 256], F32, tag="a")
                for s in range(8):
                    nc.tensor.matmul(pv[:], xts[tcH][:, s, ts * 128:(ts + 1) * 128],
                                     wv_sb[:, s, :], start=(s == 0),
                                     stop=(s == 7))
                sl = tcH * 4 + ts
                if act_evict:
                    nc.scalar.activation(
                        v_sb[:, sl, :, 0:64],
                        pv[:].rearrange("p (h e) -> p h e", e=64), AF.Copy)
                else:
                    nc.vector.tensor_copy(
                        v_sb[:, sl, :, 0:64],
                        pv[:].rearrange("p (h e) -> p h e", e=64))

            def phase_a_units(tcH, act_evict, units=None, prefetch=True):
                """Generator over phase-A units of chunk tcH (qk m-tiles and
                v tiles). act_evict routes psum evicts to Act (when the
                hosting attention chunk is PE-paced) or DVE."""
                if units is None:
                    units = [("m", 0), ("m", 1), ("m", 2), ("m", 3),
                             ("v", 0), ("v", 1), ("v", 2), ("v", 3)]
                for n, (kind, i) in enumerate(units):
                    if kind == "m":
                        qk_unit(tcH, i, act_evict)
                    else:
                        v_unit(tcH, i, act_evict)
                    if prefetch and tcH + 1 < NCH and n == 2:
                        # prefetch the next x chunk + cos/sin a chunk ahead;
                        # emitted AFTER this chunk's first repack DMAs so its
                        # ~5us SP queue hold can't delay the chunk boundary
                        nxt = slice((tcH + 1) * TCH, (tcH + 2) * TCH)
                        nc.sync.dma_start(
                            xts[tcH + 1][:],
                            xt_in.ap()[:, nxt]
                            .rearrange("(s p) m -> p s m", p=128))
                        nc.gpsimd.dma_start(cos_sb[:, nxt], cos_in[:, nxt])
                        nc.gpsimd.dma_start(sin_sb[:, nxt], sin_in[:, nxt])
                    yield

            def proj_units(tcH, yT_ch, evict_mode, alt_pool=False):
                """Generator: 8 units (one out m-tile each) of chunk tcH.
                evict_mode: 'act', 'dve', or 'alt' (alternate, for the tail).
                alt_pool (final call only): also use the now-free ps_o pool so
                the psum ring is 4 deep and the tail paces on matmuls, not
                evicts."""
                tcols = slice(tcH * TCH, (tcH + 1) * TCH)
                o_st2 = None
                for mo in range(8):
                    if alt_pool and mo % 2 == 1:
                        pp = ps_o.tile([128, TCH], F32, tag="O")
                    else:
                        pp = ps_a.tile([128, TCH], F32, tag="a")
                    for s in range(2):
                        nc.tensor.matmul(pp[:],
                                         wp_sb[:, s, mo * 128:(mo + 1) * 128],
                                         yT_ch[:, s, :], start=(s == 0),
                                         stop=(s == 1))
                    if mo % 2 == 0:
                        o_st2 = ost.tile([128, 2, TCH], BF16, tag="ost",
                                         name="o_st2")
                    use_act = (evict_mode == "act" or
                               (evict_mode == "alt" and mo % 2 == 1) or
                               (evict_mode == "tailact" and mo >= 7))
                    if use_act:
                        nc.scalar.activation(o_st2[:, mo % 2, :], pp[:],
                                             AF.Copy)
                    else:
                        nc.vector.tensor_copy(o_st2[:, mo % 2, :], pp[:])
                    if mo % 2 == 1:
                        # paired out DMA: two adjacent mo tiles per transfer
                        nc.sync.dma_start(
                            out_t[(mo - 1) * 128:(mo + 1) * 128, tcols]
                            .rearrange("(s p) m -> p s m", p=128), o_st2[:])
                    yield

            # ---- one global attention stream over (chunk, p, j) with a
            # 2-deep S window that crosses p and chunk boundaries (S has no
            # po dependency, so the pipeline never refills) ----
            seq = [(t, p, j)
                   for t in range(NCH) for p in range(2)
                   for j in range(4 * t + 4)]
            yts = {}
            pos_cur = {}

            def emit_S(t, p, j, split_exp=False):
                rr = j - 4 * t
                r = max(rr, 0) * 128
                qs = slice(t * TCH + r, (t + 1) * TCH)
                psS = ps_s.tile([128, 2, TCH], F32, tag="S")
                for hh in range(2):
                    hx = 32 * hh
                    nc.tensor.matmul(
                        psS[:, hh, r:TCH],
                        qkpk[hx:hx + 32, :, 2 * p + 1, j * 128:(j + 1) * 128],
                        qkpk[hx:hx + 32, :, 2 * p, qs],
                        start=True, stop=True, perf_mode=DR)
                expS = ep.tile([128, 2, TCH], BF16, tag="expS")
                if split_exp:
                    # final iteration: per-head exps so hh0's PV + normalize
                    # chain starts ~0.4us earlier on the critical tail
                    for hh in range(2):
                        nc.scalar.activation(expS[:, hh, r:TCH],
                                             psS[:, hh, r:TCH],
                                             AF.Exp, scale=0.125)
                else:
                    nc.scalar.activation(expS[:, :, r:TCH], psS[:, :, r:TCH],
                                         AF.Exp, scale=0.125)
                if rr >= 0:
                    for hh in range(2):
                        nc.vector.tensor_mul(expS[:, hh, r:r + 128],
                                             expS[:, hh, r:r + 128],
                                             msk_sb[:, :])
                return expS, r

            def emit_norm(t, p, hh):
                base, po = 64 * hh, pos_cur[hh]
                l_r = work.tile([1, TCH], F32R, tag="lr")
                with nc.allow_low_precision(reason="f32r rounding of 1/l"):
                    nc.vector.reciprocal(l_r[:], po[64:65, :])
                lbc = work.tile([64, TCH], F32R, tag="lbc")
                nc.gpsimd.partition_broadcast(lbc[:], l_r[:])
                nc.vector.tensor_mul(yts[t][base:base + 64, p, :],
                                     po[0:64, :], lbc[:])

            def emit_PV(t, p, j, expS, r, hh):
                nslabs = 4 * t + 4
                if j == 0:
                    pos_cur[hh] = ps_o.tile([128, TCH], F32, tag="O",
                                            name=f"po{hh}")
                nc.tensor.matmul(pos_cur[hh][0:65, r:TCH],
                                 v_sb[:, j, 2 * p + hh, :],
                                 expS[:, hh, r:TCH],
                                 start=(j == 0),
                                 stop=(j == nslabs - 1))
                if j == nslabs - 1:
                    emit_norm(t, p, hh)

            # chunk 0: only the units p=0's attention needs immediately run
            # upfront (q01, k01, v0, v1); the rest weave into attention(0)
            # just before their emission deadlines. Act is idle at startup,
            # so chunk-0 evicts ride it and the DVE rope chain stays short.
            for _ in phase_a_units(
                    0, act_evict=True,
                    units=[("m", 0), ("m", 1), ("v", 0), ("v", 1)]):
                pass
            pa0_rest = [("m", 2), ("m", 3), ("v", 2), ("v", 3)]
            # wp is not needed until proj(0), woven into attention(1); it
            # rides the SP queue behind chunk 0/1's x data
            nc.sync.dma_start(wp_sb[:],
                              wp.ap().rearrange("(s p) m -> p s m", p=128))

            chunk_iters = [2 * (4 * t + 4) for t in range(NCH)]
            chunk_start = [sum(chunk_iters[:t]) for t in range(NCH)]
            # weave entries: [gate_idx, generator]; a generator may not fire
            # before seq position gate_idx (proj(t-1) must wait for chunk
            # t-1's final normalize to be emitted)
            weave = []
            acc = [0.0]
            step = [0.0]

            def setup_chunk(t):
                yts[t] = yp.tile([128, 2, TCH], BF16, tag="yT",
                                 name=f"yT{t}")
                act_side = t <= 1
                del weave[:]
                units = 0.0
                # proj is deferred late: the final chunk is exp(Act)-
                # saturated and PE-starved, while earlier chunks host phase A
                # and are PE-bound. proj(0) fires in chunk 2's last third
                # (after phase A(3) front-loading leaves it empty).
                projs = {3: [(0, 0), (1, 0), (2, 0)]}.get(t, [])
                for tp, off in projs:
                    # mo 6/7 of each generator are the held-back units that
                    # drain in the tail, where Act is idle post-exp and DVE
                    # carries the normalize chain
                    weave.append([chunk_start[t] + off,
                                  proj_units(tp, yts[tp], "tailact")])
                    units += 8.0
                if t + 1 < NCH:
                    weave.append([0, phase_a_units(t + 1,
                                                   act_evict=act_side)])
                    units += 8.0
                acc[0] = 0.0
                iters = float(chunk_iters[t])
                if t == NCH - 1:
                    # hold ~4 units back: they fire in the final drain and
                    # keep PE busy under the last normalize chain
                    units -= 6.0
                else:
                    # front-load phase A so the next chunk's rope + repack
                    # DMA complete before its first S needs them; chunk 0 is
                    # shortest and its boundary chain is tightest
                    iters *= 0.7 if t == 0 else 0.8
                step[0] = units / iters

            def advance(idx):
                acc[0] += step[0]
                while acc[0] >= 1.0:
                    fired = False
                    for ent in weave:
                        if ent[0] > idx:
                            continue
                        try:
                            next(ent[1])
                            fired = True
                            weave.remove(ent)
                            weave.append(ent)
                            break
                        except StopIteration:
                            weave.remove(ent)
                            fired = True
                            break
                    if not fired:
                        break
                    acc[0] -= 1.0

            def drain(idx):
                stuck = False
                while weave and not stuck:
                    stuck = True
                    for ent in weave:
                        if ent[0] > idx:
                            continue
                        try:
                            next(ent[1])
                        except StopIteration:
                            weave.remove(ent)
                        stuck = False
                        break

            setup_chunk(0)
            cur = [0]
            win = [emit_S(*seq[0]), emit_S(*seq[1])]
            for idx, (t, p, j) in enumerate(seq):
                if idx < len(pa0_rest):
                    # chunk 0's deferred units: emitted here so they overlap
                    # the first S/exp chains; m2/m3 still precede the S
                    # lookahead (idx 2) that reads their rope output
                    kind, i = pa0_rest[idx]
                    if kind == "m":
                        # m2/m3 feed chunk-0 p1 attention through the rope +
                        # repack chain: route their evicts via DVE, away
                        # from the exp-backlogged Act queue
                        qk_unit(0, i, act_evict=False)
                    else:
                        v_unit(0, i, act_evict=True)
                nxt = idx + 2
                if nxt < len(seq):
                    tn = seq[nxt][0]
                    if tn > cur[0]:
                        # next chunk's S needs its rope fully woven first
                        drain(idx)
                        setup_chunk(tn)
                        cur[0] = tn
                    win.append(emit_S(*seq[nxt]))
                expS, r = win.pop(0)
                emit_PV(t, p, j, expS, r, 0)
                emit_PV(t, p, j, expS, r, 1)
                advance(idx)
            drain(len(seq))
            for _ in proj_units(NCH - 1, yts[NCH - 1], "alt", alt_pool=True):
                pass

    nc.compile()
    return nc


def _rope_tables():
    pos = np.arange(T, dtype=np.float32)[:, None]                  # [T, 1]
    i = np.arange(1, HS // 2 + 1, dtype=np.float32)[None]          # [1, 32]
    theta = 1.0 / 10000.0 ** (2.0 * (i - 1.0) / HS)
    ang = pos * theta
    cos, sin = np.cos(ang).T, np.sin(ang).T                        # [32, T]
    cos_rep = np.tile(cos, (4, 1)).astype(np.float32)              # [128, T]
    # partition layout [hA-e | hB-e | hA-o | hB-o]: odds (rows 64-127) feed
    # the even outputs with -sin; evens (rows 0-63) feed odds with +sin
    sin_sgn = np.concatenate([sin, sin, -sin, -sin], 0).astype(np.float32)
    return cos_rep, sin_sgn


def _mask128():
    p = np.arange(128)[:, None]
    f = np.arange(128)[None, :]
    return (p <= f).astype(np.float32)


def kernel(x, W_qkv, b_qkv, W_proj, b_proj):
    global last_results
    import ml_dtypes
    from concourse.bass_utils import run_bass_kernel_spmd

    bf16 = ml_dtypes.bfloat16

    if "nc" not in _cache:
        _cache["nc"] = _build()
    nc = _cache["nc"]

    x = np.asarray(x, np.float32)
    W_qkv = np.asarray(W_qkv, np.float32)
    b_qkv = np.asarray(b_qkv, np.float32)
    W_proj = np.asarray(W_proj, np.float32)
    b_proj = np.asarray(b_proj, np.float32)

    ev = np.arange(0, HS, 2)
    od = np.arange(1, HS, 2)
    cos_rep, sin_sgn = _rope_tables()
    cmask = _mask128()

    def mtile(wa, wb):
        # m-tile col order [hA-evens | hB-evens | hA-odds | hB-odds]
        return np.concatenate([wa[:, ev], wb[:, ev], wa[:, od], wb[:, od]], 1)

    def btile(ba, bb):
        return np.concatenate([ba[ev], bb[ev], ba[od], bb[od]])

    in_maps = []
    for core in range(NCORES):
        b, g = core // 4, core % 4
        heads = [4 * g + j for j in range(HPC)]
        wq = [W_qkv[:, h * 3 * HS:h * 3 * HS + HS] for h in heads]
        wk = [W_qkv[:, h * 3 * HS + HS:h * 3 * HS + 2 * HS] for h in heads]
        wv_ = [W_qkv[:, h * 3 * HS + 2 * HS:h * 3 * HS + 3 * HS] for h in heads]
        bq = [b_qkv[h * 3 * HS:h * 3 * HS + HS] for h in heads]
        bk = [b_qkv[h * 3 * HS + HS:h * 3 * HS + 2 * HS] for h in heads]
        # col-chunks: [q01 | k01 | q23 | k23]
        wqk = np.concatenate([mtile(wq[0], wq[1]), mtile(wk[0], wk[1]),
                              mtile(wq[2], wq[3]), mtile(wk[2], wk[3])],
                             axis=1)
        # pack [s*128+p, m*128+c] -> [p, (m s c)] to match the SBUF tile
        wqk_pk = wqk.reshape(8, 128, 4, 128).transpose(1, 2, 0, 3) \
                    .reshape(128, 4096)
        bqk = np.concatenate([btile(bq[0], bq[1]), btile(bk[0], bk[1]),
                              btile(bq[2], bq[3]), btile(bk[2], bk[3])])
        in_maps.append({
            "xt_in": np.ascontiguousarray(x[b].T).astype(bf16),
            "wqk": np.ascontiguousarray(wqk_pk).astype(bf16),
            "bqk_c": np.ascontiguousarray(bqk.reshape(4, 128).T),
            "wv": np.ascontiguousarray(
                np.concatenate(wv_, axis=1).reshape(8, 128, 256)
                .transpose(1, 0, 2).reshape(128, 2048)).astype(bf16),
            "wp": np.ascontiguousarray(W_proj[g * 256:(g + 1) * 256, :]).astype(bf16),
            "cos_in": cos_rep.astype(bf16),
            "sin_in": sin_sgn.astype(bf16),
            "cmask": cmask.astype(bf16),
        })

    res = run_bass_kernel_spmd(nc, in_maps, core_ids=list(range(NCORES)))
    last_results = res

    out = np.zeros((B, T, C), dtype=np.float32)
    for core in range(NCORES):
        b = core // 4
        out[b] += res.results[core]["out_t"].astype(np.float32).T
    # v-bias shifts y by exactly bv per head (sum(att) == 1), so its effect
    # on the output is the constant bv_full @ W_proj
    bv_full = np.concatenate(
        [b_qkv[h * 3 * HS + 2 * HS:h * 3 * HS + 3 * HS] for h in range(H)])
    out += (b_proj + bv_full @ W_proj)[None, None, :]
    return out

